# revision 1
# baseline (speedup 1.0000x reference)
"""DenseGrid multi-LOD trilinear embedding lookup on 8 trn2 NeuronCores.

Data-parallel over points (250k/core), codebooks replicated.  Per level:
 - L0/L1 (tables <= 32767 cells): on-device cell-major reorder of the
   codebook (8 shifted strided copies -> [cells, 8 corners, 8 feat] 256B
   entries), then ONE 256B dma_gather descriptor per point per level
   (GPSIMD extended-instruction gather, fast descriptor generation).
 - L2-4 (tables beyond dma_gather's int16 index window): x-pair gathers
   (64B: rows (x0,x0+1)) via GPSIMD indirect DMA, 4 descriptors per
   point per level.
DVE computes coords, fractional weights, flat indices, and the weighted
8-corner reduction.  dma_gather wants its int16 indices wrapped as
[j%16, j//16] while its output lands at [j%128, j//128]; the host ships
a second, statically permuted copy of the coords (ptsT3) so index tiles
are computed directly in wrapped layout.
"""
import math
import numpy as np

LODS = [16, 32, 64, 128, 256]
FEAT = 8
N_CORES = 8
P = 128
F = 64                  # points per partition per block
BLK = P * F             # 8192 points per block
NI = 512                # idxs per dma_gather call
DG_LEVELS = (0, 1)

_CACHE = {}

_CORNERS = [(a, b, c) for a in (0, 1) for b in (0, 1) for c in (0, 1)]  # (dz,dy,dx)


def _build(nblk):
    from concourse import bass, mybir
    import concourse.bacc as bacc
    import concourse.tile as tile

    npad = nblk * BLK
    nc = bacc.Bacc("TRN2", target_bir_lowering=False, debug=False,
                   num_devices=N_CORES)
    ptsT = nc.dram_tensor("ptsT", [3, npad], mybir.dt.float32, kind="ExternalInput")
    ptsT3 = nc.dram_tensor("ptsT3", [3, npad], mybir.dt.float32, kind="ExternalInput")
    cbs = [nc.dram_tensor(f"cb{i}", [LODS[i] ** 3, FEAT], mybir.dt.float32,
                          kind="ExternalInput") for i in range(5)]
    out_d = nc.dram_tensor("out", [npad, FEAT], mybir.dt.float32,
                           kind="ExternalOutput")
    f32 = mybir.dt.float32
    i16 = mybir.dt.int16
    i32 = mybir.dt.int32
    Alu = mybir.AluOpType

    cms = {}
    for lvl in DG_LEVELS:
        n1 = LODS[lvl] - 1
        cms[lvl] = nc.dram_tensor(f"cm{lvl}", [n1 ** 3, 64], f32)

    with tile.TileContext(nc) as tc:
        # cell-major tables: one strided DRAM->DRAM copy per corner
        for lvl in DG_LEVELS:
            r = LODS[lvl]
            n1 = r - 1
            for k, (dz, dy, dx) in enumerate(_CORNERS):
                for z0 in range(n1):
                    src = bass.AP(cbs[lvl], (dx + dy * r + (z0 + dz) * r * r) * 8,
                                  [[r * 8, n1], [8, n1], [1, 8]])
                    dst = bass.AP(cms[lvl], z0 * n1 * n1 * 64 + k * 8,
                                  [[n1 * 64, n1], [64, n1], [1, 8]])
                    nc.sync.dma_start(out=dst, in_=src)

        with tc.tile_pool(name="coords", bufs=2) as cpool, \
             tc.tile_pool(name="lvl", bufs=2) as lpool, \
             tc.tile_pool(name="g", bufs=2) as gpool, \
             tc.tile_pool(name="acc", bufs=2) as apool:
            for blk in range(nblk):
                n0 = blk * BLK
                xyz = []
                for i in range(3):
                    t = cpool.tile([P, F], f32, tag=f"c{i}")
                    nc.sync.dma_start(out=t[:], in_=ptsT[i, n0:n0 + BLK]
                                      .rearrange("(p f) -> p f", p=P))
                    xyz.append(t)
                xyz3 = []
                for i in range(3):
                    t = cpool.tile([16, 8 * F], f32, tag=f"c3{i}")
                    nc.sync.dma_start(out=t[:], in_=ptsT3[i, n0:n0 + BLK]
                                      .rearrange("(p f) -> p f", p=16))
                    xyz3.append(t)
                acc = apool.tile([P, F * FEAT], f32, tag="acc")
                first_mac = [True]

                def floor3(src_tiles, parts, width, s, tagp, want_frac):
                    g0, fr = [], []
                    for i in range(3):
                        xg = lpool.tile([parts, width], f32, tag=f"{tagp}xg")
                        nc.vector.tensor_scalar_mul(out=xg[:], in0=src_tiles[i][:],
                                                    scalar1=s)
                        xi = lpool.tile([parts, width], i32, tag=f"{tagp}xi")
                        nc.vector.tensor_copy(out=xi[:], in_=xg[:])
                        x0 = lpool.tile([parts, width], f32, tag=f"{tagp}x0{i}")
                        nc.vector.tensor_copy(out=x0[:], in_=xi[:])
                        d_ = lpool.tile([parts, width], f32, tag=f"{tagp}d")
                        nc.vector.tensor_sub(out=d_[:], in0=xg[:], in1=x0[:])
                        neg = lpool.tile([parts, width], f32, tag=f"{tagp}ng")
                        nc.vector.tensor_scalar(out=neg[:], in0=d_[:], scalar1=0.0,
                                                scalar2=None, op0=Alu.is_lt)
                        nc.vector.tensor_sub(out=x0[:], in0=x0[:], in1=neg[:])
                        g0.append(x0)
                        if want_frac:
                            f_ = lpool.tile([parts, width], f32, tag=f"{tagp}fr{i}")
                            nc.vector.tensor_sub(out=f_[:], in0=xg[:], in1=x0[:])
                            fr.append(f_)
                    return g0, fr

                def mac_corner(gs, w3):
                    tmp = lpool.tile([P, F * FEAT], f32, tag="tmp")
                    w3b = bass.AP(w3[:].tensor, w3[:].offset,
                                  [w3[:].ap[0], [1, F], [0, 8]])
                    nc.vector.tensor_tensor(out=tmp[:], in0=gs, in1=w3b, op=Alu.mult)
                    if first_mac[0]:
                        nc.vector.tensor_copy(out=acc[:], in_=tmp[:])
                        first_mac[0] = False
                    else:
                        nc.vector.tensor_add(out=acc[:], in0=acc[:], in1=tmp[:])

                for lvl, res in enumerate(LODS):
                    s = float(res - 1)
                    g0, fr = floor3(xyz, P, F, s, "n", True)
                    ws = []
                    for i, nm in enumerate(("wx0", "wy0", "wz0")):
                        w0 = lpool.tile([P, F], f32, tag=nm, name=nm)
                        nc.vector.tensor_scalar(out=w0[:], in0=fr[i][:], scalar1=-1.0,
                                                scalar2=-1.0, op0=Alu.mult,
                                                op1=Alu.subtract)
                        ws.append([w0, fr[i]])
                    wx, wy, wz = ws

                    if lvl in DG_LEVELS:
                        n1 = res - 1
                        g3, _ = floor3(xyz3, 16, 8 * F, s, "w", False)
                        cid = lpool.tile([16, 8 * F], f32, tag="cid")
                        nc.vector.tensor_scalar_mul(out=cid[:], in0=g3[1][:],
                                                    scalar1=float(n1))
                        nc.vector.tensor_add(out=cid[:], in0=cid[:], in1=g3[0][:])
                        tz = lpool.tile([16, 8 * F], f32, tag="tz")
                        nc.vector.tensor_scalar_mul(out=tz[:], in0=g3[2][:],
                                                    scalar1=float(n1 * n1))
                        nc.vector.tensor_add(out=cid[:], in0=cid[:], in1=tz[:])
                        itdg = lpool.tile([P, 8 * F], i16, tag="itdg")
                        nc.vector.tensor_copy(out=itdg[:16, :], in_=cid[:])
                        for gg in range(1, 8):
                            nc.sync.dma_start(out=itdg[16 * gg:16 * (gg + 1), :],
                                              in_=itdg[0:16, :])
                        gt = gpool.tile([P, 4 * F * 16], f32, tag="gt")
                        for call in range(BLK // NI):
                            w = NI // 128
                            out_ap = bass.AP(gt[:].tensor,
                                             gt[:].offset + call * w * 64,
                                             [gt[:].ap[0], [64, w], [1, 64]])
                            nc.gpsimd.dma_gather(
                                out_ap=out_ap,
                                in_ap=cms[lvl][:],
                                idxs_ap=itdg[:, call * (NI // 16):(call + 1) * (NI // 16)],
                                num_idxs=NI,
                                num_idxs_reg=NI,
                                elem_size=64,
                            )
                        w3 = lpool.tile([P, F], f32, tag="w3")
                        for k, (dz, dy, dx) in enumerate(_CORNERS):
                            wyz = lpool.tile([P, F], f32, tag="wyz")
                            nc.vector.tensor_mul(out=wyz[:], in0=wy[dy][:],
                                                 in1=wz[dz][:])
                            nc.vector.tensor_mul(out=w3[:], in0=wyz[:], in1=wx[dx][:])
                            gs = bass.AP(gt[:].tensor, gt[:].offset + k * 8,
                                         [gt[:].ap[0], [64, F], [1, 8]])
                            mac_corner(gs, w3)
                    else:
                        base = lpool.tile([P, F], f32, tag="base")
                        nc.vector.tensor_scalar_mul(out=base[:], in0=g0[1][:],
                                                    scalar1=float(res))
                        nc.vector.tensor_add(out=base[:], in0=base[:], in1=g0[0][:])
                        t2 = lpool.tile([P, F], f32, tag="t2")
                        nc.vector.tensor_scalar_mul(out=t2[:], in0=g0[2][:],
                                                    scalar1=float(res * res))
                        nc.vector.tensor_add(out=base[:], in0=base[:], in1=t2[:])

                        bi = lpool.tile([P, 4 * F], i32, tag="bi")
                        for q, (dy_, dz_) in enumerate(((0, 0), (1, 0), (0, 1), (1, 1))):
                            off = float(dy_ * res + dz_ * res * res)
                            bq = lpool.tile([P, F], f32, tag="bq")
                            nc.vector.tensor_scalar_add(out=bq[:], in0=base[:],
                                                        scalar1=off)
                            nc.vector.tensor_copy(out=bi[:, q * F:(q + 1) * F],
                                                  in_=bq[:])

                        gt = gpool.tile([P, 4 * F * 16], f32, tag="gt")
                        for col in range(4 * F):
                            nc.gpsimd.indirect_dma_start(
                                out=gt[:, col * 16:(col + 1) * 16],
                                out_offset=None,
                                in_=cbs[lvl][:],
                                in_offset=bass.IndirectOffsetOnAxis(
                                    ap=bi[:, col:col + 1], axis=0),
                            )

                        w3 = lpool.tile([P, F], f32, tag="w3")
                        for q, (dy_, dz_) in enumerate(((0, 0), (1, 0), (0, 1), (1, 1))):
                            wyz = lpool.tile([P, F], f32, tag="wyz")
                            nc.vector.tensor_mul(out=wyz[:], in0=wy[dy_][:],
                                                 in1=wz[dz_][:])
                            for dx in (0, 1):
                                nc.vector.tensor_mul(out=w3[:], in0=wyz[:],
                                                     in1=wx[dx][:])
                                gs = bass.AP(gt[:].tensor,
                                             gt[:].offset + q * F * 16 + dx * 8,
                                             [gt[:].ap[0], [16, F], [1, 8]])
                                mac_corner(gs, w3)

                nc.sync.dma_start(
                    out=out_d[n0:n0 + BLK].rearrange("(p f) e -> p (f e)", p=P),
                    in_=acc[:])
    nc.compile()
    return nc


def _get_nc(nblk):
    if nblk not in _CACHE:
        _CACHE[nblk] = _build(nblk)
    return _CACHE[nblk]


def _twist(ptsT_core, nblk):
    """ptsT3[i, b*BLK + p16*512 + t] = ptsT[i, b*BLK + (16*(t%8)+p16)*F + t//8]"""
    a = ptsT_core.reshape(3, nblk, 8, 16, F)          # [i, b, a, p16, s]
    a = a.transpose(0, 1, 3, 4, 2)                    # [i, b, p16, s, a]
    return np.ascontiguousarray(a.reshape(3, nblk * BLK))


def kernel(pts, cb0, cb1, cb2, cb3, cb4):
    from concourse.bass_utils import run_bass_kernel_spmd

    n = pts.shape[0]
    nc_pts = math.ceil(n / N_CORES)
    nblk = math.ceil(nc_pts / BLK)
    npad = nblk * BLK
    nc = _get_nc(nblk)

    cbs = {"cb0": np.ascontiguousarray(cb0, dtype=np.float32),
           "cb1": np.ascontiguousarray(cb1, dtype=np.float32),
           "cb2": np.ascontiguousarray(cb2, dtype=np.float32),
           "cb3": np.ascontiguousarray(cb3, dtype=np.float32),
           "cb4": np.ascontiguousarray(cb4, dtype=np.float32)}
    in_maps = []
    for c in range(N_CORES):
        lo = c * nc_pts
        hi = min(lo + nc_pts, n)
        p = np.full((npad, 3), 0.5, dtype=np.float32)
        p[:hi - lo] = pts[lo:hi]
        pT = np.ascontiguousarray(p.T)
        in_maps.append({"ptsT": pT, "ptsT3": _twist(pT, nblk), **cbs})

    res = run_bass_kernel_spmd(nc, in_maps, list(range(N_CORES)))
    outs = [res.results[c]["out"][:min((c + 1) * nc_pts, n) - c * nc_pts]
            for c in range(N_CORES)]
    return np.concatenate(outs, axis=0)

